# revision 28
# baseline (speedup 1.0000x reference)
"""Trainium2 Bass kernel for the 2-layer hyperbolic (Poincare ball) GCN encoder.

Strategy (8 NeuronCores, SPMD):
  - Nodes sharded across cores (2500 rows/core, padded to 2560 = 20 tiles of 128),
    with a per-core degree-balanced permutation so every 128-destination block
    has ~equal edge count (C = max chunks/block drops 23 -> 21).
  - Weights replicated (bf16); dense mobius_matvec/mobius_add/logmap0 computed on
    the owned shard with all per-row reductions fused into per-partition scalar
    "grid" tensors of shape [128, T].
  - Per-layer exchange: tangent features (pre-scaled by deg^-0.5 on the source
    side) are AllGathered in bf16 across the 8 cores.
  - Edges partitioned by destination, sorted by (block, src) for HBM gather
    locality, grouped into 128-destination blocks x 128-edge chunks. Messages
    fetched with dma_gather (1024 rows per instruction) round-robined over all
    4 SWDGE queues with 8 rotating msg buffers so descriptor generation runs
    on multiple Q7 core pairs concurrently.
  - Segment-sum on TensorE via 0/1 selection matrices (broadcast is_equal)
    accumulated in PSUM.  The per-block expmap0 epilogue is deferred and
    batched over groups of 5 blocks, so Sqrt/Tanh/Square activation-table
    reloads drop ~4x; layer-1's mobius_matvec (pass 1) is emitted inside
    layer-0's phase B for cross-layer overlap.
"""
import os
import numpy as np
import ml_dtypes

import concourse.bass as bass
import concourse.bacc as bacc
import concourse.tile as tile
import concourse.mybir as mybir
from concourse.bass_utils import run_bass_kernel_spmd
from concourse.masks import make_identity

NCORES = 8
P = 128
GN = 1024            # indices per dma_gather
CPG = GN // P        # chunks per gather
NQ = 4               # SWDGE queues
NB = 8               # rotating msg buffers (2 per queue)
MN = 1.0 - 4e-3
EPS = 1e-15
ATEPS = 1e-7

f32 = mybir.dt.float32
bf16 = mybir.dt.bfloat16
i16 = mybir.dt.int16
AF = mybir.ActivationFunctionType
OP = mybir.AluOpType

_prog_cache = {}


# ----------------------------------------------------------------- host side

def _np_expmap0(u):
    u = np.asarray(u, np.float32)
    n = max(float(np.linalg.norm(u)), EPS)
    v = (np.tanh(n) * u / n).astype(np.float32)
    nn = max(float(np.linalg.norm(v)), EPS)
    if nn > MN:
        v = (v / nn * MN).astype(np.float32)
    return v


def _host_prep(x, edge_index):
    x = np.asarray(x, np.float32)
    ei = np.asarray(edge_index)
    N, D = x.shape
    assert N % NCORES == 0
    n_loc = N // NCORES
    T = (n_loc + P - 1) // P
    n_pad = T * P
    assert NCORES * n_pad <= 32767, "indices must fit int16"

    loops = np.arange(N, dtype=ei.dtype)
    ei = np.concatenate([ei, np.stack([loops, loops])], axis=1)
    row, col = ei[0].astype(np.int64), ei[1].astype(np.int64)
    deg = np.bincount(col, minlength=N).astype(np.float32)
    dis = (deg ** -0.5).astype(np.float32)

    # --- per-core degree-balanced slot permutation -------------------------
    # slot = position in the padded 2560-row shard; block = slot // 128.
    # Snake-deal nodes (sorted by degree desc, ghosts deg 0 at the end) so
    # every block receives ~equal total degree -> C drops.
    inv_perm = np.empty((NCORES, n_loc), np.int64)   # local node -> slot
    perm = np.full((NCORES, n_pad), -1, np.int64)    # slot -> local node
    for r in range(NCORES):
        dloc = deg[r * n_loc:(r + 1) * n_loc]
        order = np.argsort(-dloc, kind="stable")     # nodes by degree desc
        # snake over T blocks, 128 rounds; ghosts fill the tail rounds
        ids = np.full(n_pad, -1, np.int64)
        ids[:n_loc] = order
        ids = ids.reshape(P, T)                      # round-major
        ids[1::2] = ids[1::2, ::-1]                  # serpentine
        for b in range(T):
            blk_nodes = ids[:, b]
            for j, nd in enumerate(blk_nodes):
                slot = b * P + j
                perm[r, slot] = nd
                if nd >= 0:
                    inv_perm[r, nd] = slot

    src_core = row // n_loc
    src_slot = inv_perm[src_core, row % n_loc]
    src_pad = src_core * n_pad + src_slot            # index into permuted ts_full
    dst_core = col // n_loc
    dst_slot = inv_perm[dst_core, col % n_loc]

    blk = dst_slot // P + dst_core * T
    # sort edges by (block, src) for gather locality
    order = np.lexsort((src_pad, blk))
    src_s = src_pad[order]
    blk_s = blk[order]
    dstrel_s = (dst_slot[order] % P).astype(np.float32)

    blk_counts = np.bincount(blk_s, minlength=NCORES * T)
    C = int(np.ceil(blk_counts.max() / P))
    NG = (T * C * P + GN - 1) // GN

    gidx = np.zeros((NCORES, P, T * C), np.int64)
    edst = np.full((NCORES, P, T * C), -1.0, np.float32)

    bounds = np.concatenate([[0], np.cumsum(blk_counts)])
    for r in range(NCORES):
        for b in range(T):
            lo, hi = bounds[r * T + b], bounds[r * T + b + 1]
            L = hi - lo
            if L == 0:
                continue
            nchunks = (L + P - 1) // P
            padded = np.zeros(nchunks * P, np.int64)
            padded[:L] = src_s[lo:hi]
            dpad = np.full(nchunks * P, -1.0, np.float32)
            dpad[:L] = dstrel_s[lo:hi]
            cols = b * C + np.arange(nchunks)
            gidx[r][:, cols] = padded.reshape(nchunks, P).T
            edst[r][:, cols] = dpad.reshape(nchunks, P).T

    # linear edge-slot order (slot j*128+p), padded to NG*GN, int16-wrapped.
    # NOTE: pad with 0 (a valid row), NOT -1 — the gather ucode trims trailing
    # negatives, which desyncs the prepare-time descriptor reservation that
    # trigger_dma later fires.
    idx_w = np.zeros((NCORES, 128, NG * (GN // 16)), np.int16)
    for r in range(NCORES):
        lin = np.zeros(NG * GN, np.int64)
        lin[:T * C * P] = gidx[r].T.ravel()
        w = lin.reshape(NG, GN // 16, 16).transpose(2, 0, 1).reshape(16, -1)
        idx_w[r] = np.tile(w.astype(np.int16), (8, 1))

    dis_loc = np.zeros((NCORES, P, T), np.float32)
    x_loc = np.zeros((NCORES, n_pad, D), np.float32)
    for r in range(NCORES):
        for slot in range(n_pad):
            nd = perm[r, slot]
            if nd >= 0:
                dis_loc[r, slot % P, slot // P] = dis[r * n_loc + nd]
                x_loc[r, slot] = x[r * n_loc + nd]

    iota = np.tile(np.arange(P, dtype=np.float32)[None, :], (P, 1))
    meta = dict(N=N, D=D, n_loc=n_loc, T=T, C=C, NG=NG, n_pad=n_pad, perm=perm)
    per_core = [dict(x=x_loc[r], dis=dis_loc[r], gidx=idx_w[r],
                     edst=edst[r].astype(ml_dtypes.bfloat16),
                     iota=iota.astype(ml_dtypes.bfloat16))
                for r in range(NCORES)]
    return meta, per_core


# --------------------------------------------------------------- device side

def _build_program(T, C, NG, DC):
    INTERLEAVE = os.environ.get("KIL", "1") == "1"
    D = DC * P
    NPAD = T * P
    EX = bf16

    nc = bacc.Bacc("TRN2", target_bir_lowering=False, debug=False,
                   num_devices=NCORES, num_swdge_queues=4,
                   dynamic_dma_scratch_size=int(os.environ.get("KSCRATCH", "16384")))

    x_in = nc.dram_tensor("x", [NPAD, D], f32, kind="ExternalInput")
    wt_in = nc.dram_tensor("wt", [2, D, D], bf16, kind="ExternalInput")
    y_in = nc.dram_tensor("y", [2, P, D], f32, kind="ExternalInput")
    iota_in = nc.dram_tensor("iota", [P, P], bf16, kind="ExternalInput")
    dis_in = nc.dram_tensor("dis", [P, T], f32, kind="ExternalInput")
    gidx_in = nc.dram_tensor("gidx", [P, NG * (GN // 16)], i16,
                             kind="ExternalInput")
    edst_in = nc.dram_tensor("edst", [P, T * C], bf16, kind="ExternalInput")
    out_ext = nc.dram_tensor("out", [NPAD, D], f32, kind="ExternalOutput")

    with tile.TileContext(nc) as tc:
        with (
            tc.tile_pool(name="const", bufs=1) as constp,
            tc.tile_pool(name="grid", bufs=1) as gridp,
            tc.tile_pool(name="big", bufs=1) as bigp,
            tc.tile_pool(name="work", bufs=3) as workp,
            tc.tile_pool(name="junk", bufs=3) as junkp,
            tc.tile_pool(name="msgs", bufs=8) as msgp,
            tc.tile_pool(name="sblk", bufs=2) as sblkp,
            tc.tile_pool(name="psum", bufs=2, space="PSUM") as psump,
            tc.tile_pool(name="psag", bufs=3, space="PSUM") as psagp,
            tc.tile_pool(name="dram", bufs=1, space="DRAM") as dramp,
        ):
            # ---- constants ----
            wt_sb = constp.tile([P, 2 * DC * D], bf16, name="wt", tag="wt")
            for l in range(2):
                for k in range(DC):
                    nc.sync.dma_start(
                        out=wt_sb[:, (l * DC + k) * D:(l * DC + k + 1) * D],
                        in_=wt_in[l, k * P:(k + 1) * P, :])
            y_sb = constp.tile([P, 2 * D], f32, name="y", tag="y")
            nc.sync.dma_start(out=y_sb[:, 0:D], in_=y_in[0])
            nc.sync.dma_start(out=y_sb[:, D:2 * D], in_=y_in[1])
            iota_sb = constp.tile([P, P], bf16, name="iota", tag="iota")
            nc.sync.dma_start(out=iota_sb[:], in_=iota_in[:, :])
            ident = constp.tile([P, P], f32, name="ident", tag="ident")
            make_identity(nc, ident[:])
            disg = constp.tile([P, T], f32, name="dis", tag="dis")
            nc.sync.dma_start(out=disg[:], in_=dis_in[:, :])
            gidx_sb = constp.tile([P, NG * (GN // 16)], i16, name="gidx",
                                  tag="gidx")
            nc.sync.dma_start(out=gidx_sb[:], in_=gidx_in[:, :])
            edst_sb = constp.tile([P, T * C], bf16, name="edst", tag="edst")
            nc.sync.dma_start(out=edst_sb[:], in_=edst_in[:, :])

            # ---- persistent big tensors ----
            h_grid = bigp.tile([P, T * D], f32, name="h", tag="h")  # h then u
            agg_grid = bigp.tile([P, T * D], bf16, name="agg", tag="agg")
            hn2 = gridp.tile([P, T], f32, name="hn2", tag="hn2")

            def G(tag):
                return gridp.tile([P, T], f32, name=tag, tag=tag)

            def tsl(t):
                return slice(t * D, (t + 1) * D)

            def artanh2(nm, xx):
                """grid of 2*artanh(clip(xx)), xx >= 0"""
                xcl = G(nm + "_xcl")
                nc.vector.tensor_scalar_min(xcl[:], xx[:], 1.0 - ATEPS)
                a1 = G(nm + "_a1")
                nc.scalar.activation(a1[:], xcl[:], AF.Ln, bias=1.0, scale=1.0)
                omx = G(nm + "_omx")
                nc.vector.tensor_scalar(out=omx[:], in0=xcl[:], scalar1=-1.0,
                                        scalar2=1.0, op0=OP.mult, op1=OP.add)
                a2 = G(nm + "_a2")
                nc.scalar.activation(a2[:], omx[:], AF.Ln)
                at2 = G(nm + "_at2")
                nc.vector.tensor_tensor(out=at2[:], in0=a1[:], in1=a2[:],
                                        op=OP.subtract)
                return at2

            def expmap_scalars(nm, n2, dis_ap):
                n = G(nm + "_n")
                nc.scalar.activation(n[:], n2[:], AF.Sqrt)
                if dis_ap is not None:
                    npr = G(nm + "_npr")
                    nc.vector.tensor_tensor(out=npr[:], in0=n[:], in1=dis_ap,
                                            op=OP.mult)
                else:
                    npr = n
                ng = G(nm + "_ng")
                nc.vector.tensor_scalar_max(ng[:], npr[:], EPS)
                tn = G(nm + "_tn")
                nc.scalar.activation(tn[:], npr[:], AF.Tanh)
                rec = G(nm + "_rec")
                nc.vector.reciprocal(rec[:], ng[:])
                sc0 = G(nm + "_sc0")
                nc.vector.tensor_tensor(out=sc0[:], in0=tn[:], in1=rec[:],
                                        op=OP.mult)
                tng = G(nm + "_tng")
                nc.vector.tensor_scalar_max(tng[:], tn[:], EPS)
                trec = G(nm + "_trec")
                nc.vector.reciprocal(trec[:], tng[:])
                ps = G(nm + "_ps")
                nc.vector.tensor_scalar(out=ps[:], in0=trec[:], scalar1=MN,
                                        scalar2=1.0, op0=OP.mult, op1=OP.min)
                sig = G(nm + "_sig")
                nc.vector.tensor_tensor(out=sig[:], in0=sc0[:], in1=ps[:],
                                        op=OP.mult)
                if dis_ap is not None:
                    sig2 = G(nm + "_sig2")
                    nc.vector.tensor_tensor(out=sig2[:], in0=sig[:],
                                            in1=dis_ap, op=OP.mult)
                    sig = sig2
                tnm = G(nm + "_tnm")
                nc.vector.tensor_scalar_min(tnm[:], tn[:], MN)
                nc.vector.tensor_tensor(out=hn2[:], in0=tnm[:], in1=tnm[:],
                                        op=OP.mult)
                return sig

            # ================= init: h = expmap0(x) =================
            n2i = G("n2i")
            for t in range(T):
                nc.sync.dma_start(out=h_grid[:, tsl(t)],
                                  in_=x_in[t * P:(t + 1) * P, :])
                jj = junkp.tile([P, D], f32, name="junk", tag="junk")
                nc.scalar.activation(jj[:], h_grid[:, tsl(t)], AF.Square,
                                     accum_out=n2i[:, t:t + 1])
            sig0 = expmap_scalars("em0", n2i, None)
            for t in range(T):
                nc.vector.tensor_scalar_mul(h_grid[:, tsl(t)],
                                            h_grid[:, tsl(t)],
                                            sig0[:, t:t + 1])

            mxn2_g = [G("mxn2_0"), G("mxn2_1")]

            def emit_pass1(l, t):
                pt = psump.tile([P, D], f32, name="pt", tag="pt")
                for k in range(DC):
                    nc.tensor.transpose(
                        out=pt[:, k * P:(k + 1) * P],
                        in_=h_grid[:, t * D + k * P: t * D + (k + 1) * P],
                        identity=ident[:])
                hT = workp.tile([P, D], bf16, name="hT", tag="hT")
                nc.vector.tensor_copy(hT[:], pt[:])
                pm = psump.tile([P, D], f32, name="pm", tag="pm")
                for k in range(DC):
                    nc.tensor.matmul(
                        pm[:],
                        lhsT=hT[:, k * P:(k + 1) * P],
                        rhs=wt_sb[:, (l * DC + k) * D:(l * DC + k + 1) * D],
                        start=(k == 0), stop=(k == DC - 1))
                nc.scalar.copy(agg_grid[:, tsl(t)], pm[:])
                jj = junkp.tile([P, D], f32, name="junk", tag="junk")
                nc.scalar.activation(jj[:], pm[:], AF.Square,
                                     accum_out=mxn2_g[l][:, t:t + 1])

            # ================= layers =================
            ts_loc = [dramp.tile([NPAD, D], EX, name="ts_loc%d" % l,
                                 tag="ts_loc%d" % l) for l in range(2)]
            ts_full = [dramp.tile([NCORES * NPAD, D], EX, addr_space="Shared",
                                  name="ts_full%d" % l, tag="ts_full%d" % l)
                       for l in range(2)]
            y2col = gridp.tile([P, 1], f32, name="y2col", tag="y2col")
            avt = {}

            def GA(tag):
                if tag not in avt:
                    avt[tag] = G(tag)
                return avt[tag]

            def artanh2h(nm, xx, cs):
                xcl = GA(nm + "_xcl")
                nc.vector.tensor_scalar_min(xcl[:, cs], xx[:, cs],
                                            1.0 - ATEPS)
                a1 = GA(nm + "_a1")
                nc.scalar.activation(a1[:, cs], xcl[:, cs], AF.Ln,
                                     bias=1.0, scale=1.0)
                omx = GA(nm + "_omx")
                nc.vector.tensor_scalar(out=omx[:, cs], in0=xcl[:, cs],
                                        scalar1=-1.0, scalar2=1.0,
                                        op0=OP.mult, op1=OP.add)
                a2 = GA(nm + "_a2")
                nc.scalar.activation(a2[:, cs], omx[:, cs], AF.Ln)
                at2 = GA(nm + "_at2")
                nc.vector.tensor_tensor(out=at2[:, cs], in0=a1[:, cs],
                                        in1=a2[:, cs], op=OP.subtract)
                return at2

            def emit_stageA(l, cs, trng, first):
                """mobius_add scalar stages + passes 2/3 + ts out for tiles
                trng of layer l (columns cs of the grid tensors)."""
                y_ap = y_sb[:, l * D:(l + 1) * D]
                mxn2 = mxn2_g[l]
                if first:
                    jy = junkp.tile([P, D], f32, name="junk", tag="junk")
                    nc.scalar.activation(jy[:], y_ap, AF.Square,
                                         accum_out=y2col[:])
                # stage 1
                xn = GA("xn")
                nc.scalar.activation(xn[:, cs], hn2[:, cs], AF.Sqrt)
                mxn = GA("mxn")
                nc.scalar.activation(mxn[:, cs], mxn2[:, cs], AF.Sqrt)
                xng = GA("xng")
                nc.vector.tensor_scalar_max(xng[:, cs], xn[:, cs], EPS)
                xrec = GA("xrec")
                nc.vector.reciprocal(xrec[:, cs], xng[:, cs])
                at2 = artanh2h("s1", xn, cs)
                rr2 = GA("rr2")
                nc.vector.tensor_tensor(out=rr2[:, cs], in0=at2[:, cs],
                                        in1=xrec[:, cs], op=OP.mult)
                mxng = GA("mxng")
                nc.vector.tensor_scalar_max(mxng[:, cs], mxn[:, cs], EPS)
                mrec = GA("mrec")
                nc.vector.reciprocal(mrec[:, cs], mxng[:, cs])
                cc = GA("cc")
                nc.vector.scalar_tensor_tensor(out=cc[:, cs],
                                               in0=mxn[:, cs], scalar=0.5,
                                               in1=rr2[:, cs],
                                               op0=OP.mult, op1=OP.mult)
                tch = GA("tch")
                nc.scalar.activation(tch[:, cs], cc[:, cs], AF.Tanh)
                tcg = GA("tcg")
                nc.vector.tensor_scalar_max(tcg[:, cs], tch[:, cs], EPS)
                tcrec = GA("tcrec")
                nc.vector.reciprocal(tcrec[:, cs], tcg[:, cs])
                psA = GA("psA")
                nc.vector.tensor_scalar(out=psA[:, cs], in0=tcrec[:, cs],
                                        scalar1=MN, scalar2=1.0,
                                        op0=OP.mult, op1=OP.min)
                sp0 = GA("sp0")
                nc.vector.tensor_tensor(out=sp0[:, cs], in0=tch[:, cs],
                                        in1=mrec[:, cs], op=OP.mult)
                spg = GA("spg")
                nc.vector.tensor_tensor(out=spg[:, cs], in0=sp0[:, cs],
                                        in1=psA[:, cs], op=OP.mult)
                tcm = GA("tcm")
                nc.vector.tensor_scalar_min(tcm[:, cs], tch[:, cs], MN)
                x2 = GA("x2")
                nc.vector.tensor_tensor(out=x2[:, cs], in0=tcm[:, cs],
                                        in1=tcm[:, cs], op=OP.mult)
                # pass 2: xy = sum((sp*mx) . y)
                xy = GA("xy")
                for t in trng:
                    jx = junkp.tile([P, D], f32, name="junk", tag="junk")
                    nc.vector.scalar_tensor_tensor(
                        out=jx[:], in0=agg_grid[:, tsl(t)],
                        scalar=spg[:, t:t + 1], in1=y_ap,
                        op0=OP.mult, op1=OP.mult,
                        accum_out=xy[:, t:t + 1])
                # stage 2
                t0 = GA("t0")
                nc.vector.tensor_scalar(out=t0[:, cs], in0=xy[:, cs],
                                        scalar1=2.0, scalar2=1.0,
                                        op0=OP.mult, op1=OP.add)
                ag = GA("ag")
                nc.vector.tensor_scalar_add(ag[:, cs], t0[:, cs],
                                            y2col[:, 0:1])
                d0 = GA("d0")
                nc.vector.tensor_scalar_mul(d0[:, cs], x2[:, cs],
                                            y2col[:, 0:1])
                d1 = GA("d1")
                nc.vector.tensor_tensor(out=d1[:, cs], in0=d0[:, cs],
                                        in1=t0[:, cs], op=OP.add)
                dg = GA("dg")
                nc.vector.tensor_scalar_max(dg[:, cs], d1[:, cs], EPS)
                dinv = GA("dinv")
                nc.vector.reciprocal(dinv[:, cs], dg[:, cs])
                alpha = GA("alpha")
                nc.vector.tensor_tensor(out=alpha[:, cs], in0=ag[:, cs],
                                        in1=dinv[:, cs], op=OP.mult)
                bsc = GA("bsc")
                nc.vector.tensor_scalar(out=bsc[:, cs], in0=x2[:, cs],
                                        scalar1=-1.0, scalar2=1.0,
                                        op0=OP.mult, op1=OP.add)
                beta = GA("beta")
                nc.vector.tensor_tensor(out=beta[:, cs], in0=bsc[:, cs],
                                        in1=dinv[:, cs], op=OP.mult)
                alphasp = GA("alphasp")
                nc.vector.tensor_tensor(out=alphasp[:, cs],
                                        in0=alpha[:, cs], in1=spg[:, cs],
                                        op=OP.mult)
                # pass 3: u = alphasp*mx + beta*y (into h_grid)
                un2 = GA("un2")
                for t in trng:
                    t1 = workp.tile([P, D], f32, name="t1", tag="t1")
                    nc.vector.tensor_scalar_mul(t1[:], y_ap,
                                                beta[:, t:t + 1])
                    us = h_grid[:, tsl(t)]
                    nc.vector.scalar_tensor_tensor(
                        out=us, in0=agg_grid[:, tsl(t)],
                        scalar=alphasp[:, t:t + 1], in1=t1[:],
                        op0=OP.mult, op1=OP.add)
                    ju = junkp.tile([P, D], f32, name="junk", tag="junk")
                    nc.scalar.activation(ju[:], us, AF.Square,
                                         accum_out=un2[:, t:t + 1])
                # stage 3: gamma
                un = GA("un")
                nc.scalar.activation(un[:, cs], un2[:, cs], AF.Sqrt)
                ung = GA("ung")
                nc.vector.tensor_scalar_max(ung[:, cs], un[:, cs], EPS)
                urec = GA("urec")
                nc.vector.reciprocal(urec[:, cs], ung[:, cs])
                h2n = GA("h2n")
                nc.vector.tensor_scalar_min(h2n[:, cs], un[:, cs], MN)
                at2u = artanh2h("s3", h2n, cs)
                h2ng = GA("h2ng")
                nc.vector.tensor_scalar_max(h2ng[:, cs], h2n[:, cs], EPS)
                hrec = GA("hrec")
                nc.vector.reciprocal(hrec[:, cs], h2ng[:, cs])
                lam2 = GA("lam2")
                nc.vector.tensor_tensor(out=lam2[:, cs], in0=at2u[:, cs],
                                        in1=hrec[:, cs], op=OP.mult)
                pst = GA("pst")
                nc.vector.tensor_scalar(out=pst[:, cs], in0=urec[:, cs],
                                        scalar1=MN, scalar2=1.0,
                                        op0=OP.mult, op1=OP.min)
                gm0 = GA("gm0")
                nc.vector.scalar_tensor_tensor(out=gm0[:, cs],
                                               in0=lam2[:, cs], scalar=0.5,
                                               in1=pst[:, cs],
                                               op0=OP.mult, op1=OP.mult)
                gam = GA("gam")
                nc.vector.tensor_tensor(out=gam[:, cs], in0=gm0[:, cs],
                                        in1=disg[:, cs], op=OP.mult)
                # ts tiles out
                for t in trng:
                    tst = workp.tile([P, D], EX, name="tst", tag="tst")
                    nc.vector.tensor_scalar_mul(tst[:],
                                                h_grid[:, tsl(t)],
                                                gam[:, t:t + 1])
                    nc.sync.dma_start(out=ts_loc[l][t * P:(t + 1) * P, :],
                                      in_=tst[:])

            def emit_AG(l):
                nc.gpsimd.collective_compute(
                    "AllGather", OP.bypass,
                    replica_groups=[list(range(NCORES))],
                    ins=[ts_loc[l].opt()], outs=[ts_full[l].opt()])

            bvt = {}

            def GB(tag):
                if tag not in bvt:
                    bvt[tag] = G(tag)
                return bvt[tag]

            def expmap_grid_cs(nm, n2, cs, with_dis=True):
                """sig2 columns cs of expmap0(dis*agg) incl. dst-side dis
                (or plain expmap0 scaling when with_dis=False);
                also writes hn2[:, cs]."""
                n = GB(nm + "_n")
                nc.scalar.activation(n[:, cs], n2[:, cs], AF.Sqrt)
                if with_dis:
                    npr = GB(nm + "_npr")
                    nc.vector.tensor_tensor(out=npr[:, cs], in0=n[:, cs],
                                            in1=disg[:, cs], op=OP.mult)
                else:
                    npr = n
                ng = GB(nm + "_ng")
                nc.vector.tensor_scalar_max(ng[:, cs], npr[:, cs], EPS)
                tn = GB(nm + "_tn")
                nc.scalar.activation(tn[:, cs], npr[:, cs], AF.Tanh)
                rec = GB(nm + "_rec")
                nc.vector.reciprocal(rec[:, cs], ng[:, cs])
                sc0 = GB(nm + "_sc0")
                nc.vector.tensor_tensor(out=sc0[:, cs], in0=tn[:, cs],
                                        in1=rec[:, cs], op=OP.mult)
                tng = GB(nm + "_tng")
                nc.vector.tensor_scalar_max(tng[:, cs], tn[:, cs], EPS)
                trec = GB(nm + "_trec")
                nc.vector.reciprocal(trec[:, cs], tng[:, cs])
                ps = GB(nm + "_ps")
                nc.vector.tensor_scalar(out=ps[:, cs], in0=trec[:, cs],
                                        scalar1=MN, scalar2=1.0,
                                        op0=OP.mult, op1=OP.min)
                sig = GB(nm + "_sig")
                nc.vector.tensor_tensor(out=sig[:, cs], in0=sc0[:, cs],
                                        in1=ps[:, cs], op=OP.mult)
                if with_dis:
                    sig2 = GB(nm + "_sig2")
                    nc.vector.tensor_tensor(out=sig2[:, cs], in0=sig[:, cs],
                                            in1=disg[:, cs], op=OP.mult)
                    sig = sig2
                tnm = GB(nm + "_tnm")
                nc.vector.tensor_scalar_min(tnm[:, cs], tn[:, cs], MN)
                nc.vector.tensor_tensor(out=hn2[:, cs], in0=tnm[:, cs],
                                        in1=tnm[:, cs], op=OP.mult)
                return sig

            GRP = 5  # blocks per epilogue group (activation-table batching)

            def emit_phaseB(l):
                an2 = G("an2")
                mtiles = {}
                for b in range(T):
                    S = sblkp.tile([P, C * P], EX, name="S", tag="S")
                    nc.vector.tensor_tensor(
                        out=S[:].rearrange("p (c j) -> p c j", c=C),
                        in0=edst_sb[:, b * C:(b + 1) * C].to_broadcast(
                            [P, C, P]),
                        in1=iota_sb[:].rearrange("p (o j) -> p o j", o=1)
                            .to_broadcast([P, C, P]),
                        op=OP.is_equal)
                    pa = psagp.tile([P, D], f32, name="pa", tag="pa")
                    for c in range(C):
                        j = b * C + c
                        g, s = divmod(j, CPG)
                        if g not in mtiles:
                            m = msgp.tile([P, CPG * D], EX, name="m", tag="m")
                            nc.gpsimd.dma_gather(
                                m[:].rearrange("p (c e) -> p c e", c=CPG),
                                ts_full[l],
                                gidx_sb[:, g * (GN // 16):(g + 1) * (GN // 16)],
                                GN, GN, D, queue_num=g % 4)
                            mtiles = {g: m}
                        m = mtiles[g]
                        nc.tensor.matmul(pa[:],
                                         lhsT=S[:, c * P:(c + 1) * P],
                                         rhs=m[:, s * D:(s + 1) * D],
                                         start=(c == 0), stop=(c == C - 1))
                    jj = junkp.tile([P, D], f32, name="junk", tag="junk")
                    nc.scalar.activation(jj[:], pa[:], AF.Square,
                                         accum_out=an2[:, b:b + 1])
                    # defer expmap scaling to the group epilogue
                    nc.scalar.copy(h_grid[:, tsl(b)], pa[:])
                    if l == 1 and b >= T - GRP:
                        # last output group: per-block epilogue so the final
                        # output DMAs stream out instead of bunching in the
                        # kernel's drain tail
                        sig = expmap_grid_cs("emB", an2, slice(b, b + 1))
                        nc.vector.tensor_scalar_mul(
                            h_grid[:, tsl(b)], h_grid[:, tsl(b)],
                            sig[:, b:b + 1])
                        nc.sync.dma_start(out=out_ext[b * P:(b + 1) * P, :],
                                          in_=h_grid[:, tsl(b)])
                    elif (b + 1) % GRP == 0:
                        g0 = b + 1 - GRP
                        cs = slice(g0, b + 1)
                        sig = expmap_grid_cs("emB", an2, cs)
                        for t in range(g0, b + 1):
                            nc.vector.tensor_scalar_mul(
                                h_grid[:, tsl(t)], h_grid[:, tsl(t)],
                                sig[:, t:t + 1])
                        if l == 0:
                            for t in range(g0, b + 1):
                                emit_pass1(1, t)
                        else:
                            for t in range(g0, b + 1):
                                nc.sync.dma_start(
                                    out=out_ext[t * P:(t + 1) * P, :],
                                    in_=h_grid[:, tsl(t)])

            # layer 0 phase A
            for t in range(T):
                emit_pass1(0, t)
            NH = 2 if T % 2 == 0 and T >= 2 else 1
            TH = T // NH
            for hh in range(NH):
                emit_stageA(0, slice(hh * TH, (hh + 1) * TH),
                            range(hh * TH, (hh + 1) * TH), first=(hh == 0))
            emit_AG(0)
            # layer 0 phase B (embeds layer-1 pass1 via group epilogues)
            emit_phaseB(0)
            # layer 1 phase A stages + exchange
            for hh in range(NH):
                emit_stageA(1, slice(hh * TH, (hh + 1) * TH),
                            range(hh * TH, (hh + 1) * TH), first=(hh == 0))
            emit_AG(1)
            # layer 1 phase B
            emit_phaseB(1)

    nc.compile()
    return nc


def _get_program(T, C, NG, DC):
    key = (T, C, NG, DC)
    if key not in _prog_cache:
        _prog_cache[key] = _build_program(T, C, NG, DC)
    return _prog_cache[key]


# ----------------------------------------------------------------- entry

def run(inputs, trace=False, trace_kwargs=None):
    x = np.asarray(inputs["x"], np.float32)
    ei = np.asarray(inputs["edge_index"])
    W1 = np.asarray(inputs["W1"], np.float32)
    b1 = np.asarray(inputs["b1"], np.float32)
    W2 = np.asarray(inputs["W2"], np.float32)
    b2 = np.asarray(inputs["b2"], np.float32)
    N, D = x.shape
    assert D % P == 0
    meta, per_core = _host_prep(x, ei)
    T, C, NG, DC = meta["T"], meta["C"], meta["NG"], D // P
    n_loc, perm = meta["n_loc"], meta["perm"]

    wt = np.stack([np.ascontiguousarray(W1.T), np.ascontiguousarray(W2.T)])
    wt = wt.astype(ml_dtypes.bfloat16)
    y = np.stack([np.tile(_np_expmap0(b1)[None, :], (P, 1)),
                  np.tile(_np_expmap0(b2)[None, :], (P, 1))])

    nc = _get_program(T, C, NG, DC)
    in_maps = []
    for r in range(NCORES):
        m = dict(per_core[r])
        m["wt"] = wt
        m["y"] = y
        in_maps.append(m)

    kwargs = {}
    if trace:
        kwargs = dict(trace=True, trace_kwargs=trace_kwargs or {})
    res = run_bass_kernel_spmd(nc, in_maps, list(range(NCORES)), **kwargs)
    out = np.empty((N, D), np.float32)
    for r in range(NCORES):
        res_r = np.asarray(res.results[r]["out"])
        pr = perm[r]
        valid = pr >= 0
        out[r * n_loc + pr[valid]] = res_r[np.nonzero(valid)[0]]
    return out, res


def kernel(**inputs):
    out, _ = run(inputs)
    return out



# revision 30
# speedup vs baseline: 1.0068x; 1.0068x over previous
"""Trainium2 Bass kernel for the 2-layer hyperbolic (Poincare ball) GCN encoder.

Strategy (8 NeuronCores, SPMD):
  - Nodes sharded across cores (2500 rows/core, padded to 2560 = 20 tiles of 128),
    with a per-core degree-balanced permutation so every 128-destination block
    has ~equal edge count (C = max chunks/block drops 23 -> 21).
  - Weights replicated (bf16); dense mobius_matvec/mobius_add/logmap0 computed on
    the owned shard with all per-row reductions fused into per-partition scalar
    "grid" tensors of shape [128, T].
  - Per-layer exchange: tangent features (pre-scaled by deg^-0.5 on the source
    side) are AllGathered in bf16 across the 8 cores.
  - Edges partitioned by destination, sorted by (block, src) for HBM gather
    locality, grouped into 128-destination blocks x 128-edge chunks. Messages
    fetched with dma_gather (1024 rows per instruction) round-robined over all
    4 SWDGE queues with 8 rotating msg buffers so descriptor generation runs
    on multiple Q7 core pairs concurrently.
  - Segment-sum on TensorE via 0/1 selection matrices (broadcast is_equal)
    accumulated in PSUM.  The per-block expmap0 epilogue is deferred and
    batched over groups of 5 blocks, so Sqrt/Tanh/Square activation-table
    reloads drop ~4x; layer-1's mobius_matvec (pass 1) is emitted inside
    layer-0's phase B for cross-layer overlap.
"""
import os
import numpy as np
import ml_dtypes

import concourse.bass as bass
import concourse.bacc as bacc
import concourse.tile as tile
import concourse.mybir as mybir
from concourse.bass_utils import run_bass_kernel_spmd
from concourse.masks import make_identity

NCORES = 8
P = 128
GN = 1024            # indices per dma_gather
CPG = GN // P        # chunks per gather
NQ = 4               # SWDGE queues
NB = 8               # rotating msg buffers (2 per queue)
MN = 1.0 - 4e-3
EPS = 1e-15
ATEPS = 1e-7

f32 = mybir.dt.float32
bf16 = mybir.dt.bfloat16
i16 = mybir.dt.int16
AF = mybir.ActivationFunctionType
OP = mybir.AluOpType

_prog_cache = {}


# ----------------------------------------------------------------- host side

def _np_expmap0(u):
    u = np.asarray(u, np.float32)
    n = max(float(np.linalg.norm(u)), EPS)
    v = (np.tanh(n) * u / n).astype(np.float32)
    nn = max(float(np.linalg.norm(v)), EPS)
    if nn > MN:
        v = (v / nn * MN).astype(np.float32)
    return v


def _host_prep(x, edge_index):
    x = np.asarray(x, np.float32)
    ei = np.asarray(edge_index)
    N, D = x.shape
    assert N % NCORES == 0
    n_loc = N // NCORES
    T = (n_loc + P - 1) // P
    n_pad = T * P
    assert NCORES * n_pad <= 32767, "indices must fit int16"

    loops = np.arange(N, dtype=ei.dtype)
    ei = np.concatenate([ei, np.stack([loops, loops])], axis=1)
    row, col = ei[0].astype(np.int64), ei[1].astype(np.int64)
    deg = np.bincount(col, minlength=N).astype(np.float32)
    dis = (deg ** -0.5).astype(np.float32)

    # --- per-core degree-balanced slot permutation -------------------------
    # slot = position in the padded 2560-row shard; block = slot // 128.
    # Snake-deal nodes (sorted by degree desc, ghosts deg 0 at the end) so
    # every block receives ~equal total degree -> C drops.
    inv_perm = np.empty((NCORES, n_loc), np.int64)   # local node -> slot
    perm = np.full((NCORES, n_pad), -1, np.int64)    # slot -> local node
    for r in range(NCORES):
        dloc = deg[r * n_loc:(r + 1) * n_loc]
        order = np.argsort(-dloc, kind="stable")     # nodes by degree desc
        # snake over T blocks, 128 rounds; ghosts fill the tail rounds
        ids = np.full(n_pad, -1, np.int64)
        ids[:n_loc] = order
        ids = ids.reshape(P, T)                      # round-major
        ids[1::2] = ids[1::2, ::-1]                  # serpentine
        for b in range(T):
            blk_nodes = ids[:, b]
            for j, nd in enumerate(blk_nodes):
                slot = b * P + j
                perm[r, slot] = nd
                if nd >= 0:
                    inv_perm[r, nd] = slot

    src_core = row // n_loc
    src_slot = inv_perm[src_core, row % n_loc]
    src_pad = src_core * n_pad + src_slot            # index into permuted ts_full
    dst_core = col // n_loc
    dst_slot = inv_perm[dst_core, col % n_loc]

    blk = dst_slot // P + dst_core * T
    # sort edges by (block, src) for gather locality
    order = np.lexsort((src_pad, blk))
    src_s = src_pad[order]
    blk_s = blk[order]
    dstrel_s = (dst_slot[order] % P).astype(np.float32)

    blk_counts = np.bincount(blk_s, minlength=NCORES * T)
    C = int(np.ceil(blk_counts.max() / P))
    NG = (T * C * P + GN - 1) // GN

    gidx = np.zeros((NCORES, P, T * C), np.int64)
    edst = np.full((NCORES, P, T * C), -1.0, np.float32)

    bounds = np.concatenate([[0], np.cumsum(blk_counts)])
    for r in range(NCORES):
        for b in range(T):
            lo, hi = bounds[r * T + b], bounds[r * T + b + 1]
            L = hi - lo
            if L == 0:
                continue
            nchunks = (L + P - 1) // P
            padded = np.zeros(nchunks * P, np.int64)
            padded[:L] = src_s[lo:hi]
            dpad = np.full(nchunks * P, -1.0, np.float32)
            dpad[:L] = dstrel_s[lo:hi]
            cols = b * C + np.arange(nchunks)
            gidx[r][:, cols] = padded.reshape(nchunks, P).T
            edst[r][:, cols] = dpad.reshape(nchunks, P).T

    # linear edge-slot order (slot j*128+p), padded to NG*GN, int16-wrapped.
    # NOTE: pad with 0 (a valid row), NOT -1 — the gather ucode trims trailing
    # negatives, which desyncs the prepare-time descriptor reservation that
    # trigger_dma later fires.
    idx_w = np.zeros((NCORES, 128, NG * (GN // 16)), np.int16)
    for r in range(NCORES):
        lin = np.zeros(NG * GN, np.int64)
        lin[:T * C * P] = gidx[r].T.ravel()
        w = lin.reshape(NG, GN // 16, 16).transpose(2, 0, 1).reshape(16, -1)
        idx_w[r] = np.tile(w.astype(np.int16), (8, 1))

    dis_loc = np.zeros((NCORES, P, T), np.float32)
    x_loc = np.zeros((NCORES, n_pad, D), np.float32)
    for r in range(NCORES):
        for slot in range(n_pad):
            nd = perm[r, slot]
            if nd >= 0:
                dis_loc[r, slot % P, slot // P] = dis[r * n_loc + nd]
                x_loc[r, slot] = x[r * n_loc + nd]

    iota = np.tile(np.arange(P, dtype=np.float32)[None, :], (P, 1))
    meta = dict(N=N, D=D, n_loc=n_loc, T=T, C=C, NG=NG, n_pad=n_pad, perm=perm)
    per_core = [dict(x=x_loc[r], dis=dis_loc[r], gidx=idx_w[r],
                     edst=edst[r].astype(ml_dtypes.bfloat16),
                     iota=iota.astype(ml_dtypes.bfloat16))
                for r in range(NCORES)]
    return meta, per_core


# --------------------------------------------------------------- device side

def _build_program(T, C, NG, DC):
    INTERLEAVE = os.environ.get("KIL", "1") == "1"
    D = DC * P
    NPAD = T * P
    EX = bf16

    nc = bacc.Bacc("TRN2", target_bir_lowering=False, debug=False,
                   num_devices=NCORES, num_swdge_queues=4,
                   dynamic_dma_scratch_size=int(os.environ.get("KSCRATCH", "16384")))

    x_in = nc.dram_tensor("x", [NPAD, D], f32, kind="ExternalInput")
    wt_in = nc.dram_tensor("wt", [2, D, D], bf16, kind="ExternalInput")
    y_in = nc.dram_tensor("y", [2, P, D], f32, kind="ExternalInput")
    iota_in = nc.dram_tensor("iota", [P, P], bf16, kind="ExternalInput")
    dis_in = nc.dram_tensor("dis", [P, T], f32, kind="ExternalInput")
    gidx_in = nc.dram_tensor("gidx", [P, NG * (GN // 16)], i16,
                             kind="ExternalInput")
    edst_in = nc.dram_tensor("edst", [P, T * C], bf16, kind="ExternalInput")
    out_ext = nc.dram_tensor("out", [NPAD, D], f32, kind="ExternalOutput")

    with tile.TileContext(nc) as tc:
        with (
            tc.tile_pool(name="const", bufs=1) as constp,
            tc.tile_pool(name="grid", bufs=1) as gridp,
            tc.tile_pool(name="big", bufs=1) as bigp,
            tc.tile_pool(name="work", bufs=3) as workp,
            tc.tile_pool(name="junk", bufs=3) as junkp,
            tc.tile_pool(name="msgs", bufs=8) as msgp,
            tc.tile_pool(name="sblk", bufs=2) as sblkp,
            tc.tile_pool(name="psum", bufs=2, space="PSUM") as psump,
            tc.tile_pool(name="psag", bufs=3, space="PSUM") as psagp,
            tc.tile_pool(name="dram", bufs=1, space="DRAM") as dramp,
        ):
            # ---- constants ----
            wt_sb = constp.tile([P, 2 * DC * D], bf16, name="wt", tag="wt")
            for l in range(2):
                for k in range(DC):
                    nc.sync.dma_start(
                        out=wt_sb[:, (l * DC + k) * D:(l * DC + k + 1) * D],
                        in_=wt_in[l, k * P:(k + 1) * P, :])
            y_sb = constp.tile([P, 2 * D], f32, name="y", tag="y")
            nc.sync.dma_start(out=y_sb[:, 0:D], in_=y_in[0])
            nc.sync.dma_start(out=y_sb[:, D:2 * D], in_=y_in[1])
            iota_sb = constp.tile([P, P], bf16, name="iota", tag="iota")
            nc.sync.dma_start(out=iota_sb[:], in_=iota_in[:, :])
            ident = constp.tile([P, P], f32, name="ident", tag="ident")
            make_identity(nc, ident[:])
            disg = constp.tile([P, T], f32, name="dis", tag="dis")
            nc.sync.dma_start(out=disg[:], in_=dis_in[:, :])
            gidx_sb = constp.tile([P, NG * (GN // 16)], i16, name="gidx",
                                  tag="gidx")
            nc.sync.dma_start(out=gidx_sb[:], in_=gidx_in[:, :])
            edst_sb = constp.tile([P, T * C], bf16, name="edst", tag="edst")
            nc.sync.dma_start(out=edst_sb[:], in_=edst_in[:, :])

            # ---- persistent big tensors ----
            h_grid = bigp.tile([P, T * D], f32, name="h", tag="h")  # h then u
            agg_grid = bigp.tile([P, T * D], bf16, name="agg", tag="agg")
            hn2 = gridp.tile([P, T], f32, name="hn2", tag="hn2")

            def G(tag):
                return gridp.tile([P, T], f32, name=tag, tag=tag)

            def tsl(t):
                return slice(t * D, (t + 1) * D)

            def artanh2(nm, xx):
                """grid of 2*artanh(clip(xx)), xx >= 0"""
                xcl = G(nm + "_xcl")
                nc.vector.tensor_scalar_min(xcl[:], xx[:], 1.0 - ATEPS)
                a1 = G(nm + "_a1")
                nc.scalar.activation(a1[:], xcl[:], AF.Ln, bias=1.0, scale=1.0)
                omx = G(nm + "_omx")
                nc.vector.tensor_scalar(out=omx[:], in0=xcl[:], scalar1=-1.0,
                                        scalar2=1.0, op0=OP.mult, op1=OP.add)
                a2 = G(nm + "_a2")
                nc.scalar.activation(a2[:], omx[:], AF.Ln)
                at2 = G(nm + "_at2")
                nc.vector.tensor_tensor(out=at2[:], in0=a1[:], in1=a2[:],
                                        op=OP.subtract)
                return at2

            def expmap_scalars(nm, n2, dis_ap):
                n = G(nm + "_n")
                nc.scalar.activation(n[:], n2[:], AF.Sqrt)
                if dis_ap is not None:
                    npr = G(nm + "_npr")
                    nc.vector.tensor_tensor(out=npr[:], in0=n[:], in1=dis_ap,
                                            op=OP.mult)
                else:
                    npr = n
                ng = G(nm + "_ng")
                nc.vector.tensor_scalar_max(ng[:], npr[:], EPS)
                tn = G(nm + "_tn")
                nc.scalar.activation(tn[:], npr[:], AF.Tanh)
                rec = G(nm + "_rec")
                nc.vector.reciprocal(rec[:], ng[:])
                sc0 = G(nm + "_sc0")
                nc.vector.tensor_tensor(out=sc0[:], in0=tn[:], in1=rec[:],
                                        op=OP.mult)
                tng = G(nm + "_tng")
                nc.vector.tensor_scalar_max(tng[:], tn[:], EPS)
                trec = G(nm + "_trec")
                nc.vector.reciprocal(trec[:], tng[:])
                ps = G(nm + "_ps")
                nc.vector.tensor_scalar(out=ps[:], in0=trec[:], scalar1=MN,
                                        scalar2=1.0, op0=OP.mult, op1=OP.min)
                sig = G(nm + "_sig")
                nc.vector.tensor_tensor(out=sig[:], in0=sc0[:], in1=ps[:],
                                        op=OP.mult)
                if dis_ap is not None:
                    sig2 = G(nm + "_sig2")
                    nc.vector.tensor_tensor(out=sig2[:], in0=sig[:],
                                            in1=dis_ap, op=OP.mult)
                    sig = sig2
                tnm = G(nm + "_tnm")
                nc.vector.tensor_scalar_min(tnm[:], tn[:], MN)
                nc.vector.tensor_tensor(out=hn2[:], in0=tnm[:], in1=tnm[:],
                                        op=OP.mult)
                return sig

            # ================= init: h = expmap0(x) =================
            n2i = G("n2i")
            for t in range(T):
                nc.sync.dma_start(out=h_grid[:, tsl(t)],
                                  in_=x_in[t * P:(t + 1) * P, :])
                jj = junkp.tile([P, D], f32, name="junk", tag="junk")
                nc.scalar.activation(jj[:], h_grid[:, tsl(t)], AF.Square,
                                     accum_out=n2i[:, t:t + 1])
            sig0 = expmap_scalars("em0", n2i, None)
            for t in range(T):
                nc.vector.tensor_scalar_mul(h_grid[:, tsl(t)],
                                            h_grid[:, tsl(t)],
                                            sig0[:, t:t + 1])

            mxn2_g = [G("mxn2_0"), G("mxn2_1")]

            def emit_pass1(l, t):
                pt = psump.tile([P, D], f32, name="pt", tag="pt")
                for k in range(DC):
                    nc.tensor.transpose(
                        out=pt[:, k * P:(k + 1) * P],
                        in_=h_grid[:, t * D + k * P: t * D + (k + 1) * P],
                        identity=ident[:])
                hT = workp.tile([P, D], bf16, name="hT", tag="hT")
                nc.vector.tensor_copy(hT[:], pt[:])
                pm = psump.tile([P, D], f32, name="pm", tag="pm")
                for k in range(DC):
                    nc.tensor.matmul(
                        pm[:],
                        lhsT=hT[:, k * P:(k + 1) * P],
                        rhs=wt_sb[:, (l * DC + k) * D:(l * DC + k + 1) * D],
                        start=(k == 0), stop=(k == DC - 1))
                nc.scalar.copy(agg_grid[:, tsl(t)], pm[:])
                jj = junkp.tile([P, D], f32, name="junk", tag="junk")
                nc.scalar.activation(jj[:], pm[:], AF.Square,
                                     accum_out=mxn2_g[l][:, t:t + 1])

            # ================= layers =================
            ts_loc = [dramp.tile([NPAD, D], EX, name="ts_loc%d" % l,
                                 tag="ts_loc%d" % l) for l in range(2)]
            ts_full = [dramp.tile([NCORES * NPAD, D], EX, addr_space="Shared",
                                  name="ts_full%d" % l, tag="ts_full%d" % l)
                       for l in range(2)]
            y2col = gridp.tile([P, 1], f32, name="y2col", tag="y2col")
            avt = {}

            def GA(tag):
                if tag not in avt:
                    avt[tag] = G(tag)
                return avt[tag]

            def artanh2h(nm, xx, cs):
                xcl = GA(nm + "_xcl")
                nc.vector.tensor_scalar_min(xcl[:, cs], xx[:, cs],
                                            1.0 - ATEPS)
                a1 = GA(nm + "_a1")
                nc.scalar.activation(a1[:, cs], xcl[:, cs], AF.Ln,
                                     bias=1.0, scale=1.0)
                omx = GA(nm + "_omx")
                nc.vector.tensor_scalar(out=omx[:, cs], in0=xcl[:, cs],
                                        scalar1=-1.0, scalar2=1.0,
                                        op0=OP.mult, op1=OP.add)
                a2 = GA(nm + "_a2")
                nc.scalar.activation(a2[:, cs], omx[:, cs], AF.Ln)
                at2 = GA(nm + "_at2")
                nc.vector.tensor_tensor(out=at2[:, cs], in0=a1[:, cs],
                                        in1=a2[:, cs], op=OP.subtract)
                return at2

            def emit_stageA(l, cs, trng, first):
                """mobius_add scalar stages + passes 2/3 + ts out for tiles
                trng of layer l (columns cs of the grid tensors)."""
                y_ap = y_sb[:, l * D:(l + 1) * D]
                mxn2 = mxn2_g[l]
                if first:
                    jy = junkp.tile([P, D], f32, name="junk", tag="junk")
                    nc.scalar.activation(jy[:], y_ap, AF.Square,
                                         accum_out=y2col[:])
                # stage 1
                xn = GA("xn")
                nc.scalar.activation(xn[:, cs], hn2[:, cs], AF.Sqrt)
                mxn = GA("mxn")
                nc.scalar.activation(mxn[:, cs], mxn2[:, cs], AF.Sqrt)
                xng = GA("xng")
                nc.vector.tensor_scalar_max(xng[:, cs], xn[:, cs], EPS)
                xrec = GA("xrec")
                nc.vector.reciprocal(xrec[:, cs], xng[:, cs])
                at2 = artanh2h("s1", xn, cs)
                rr2 = GA("rr2")
                nc.vector.tensor_tensor(out=rr2[:, cs], in0=at2[:, cs],
                                        in1=xrec[:, cs], op=OP.mult)
                mxng = GA("mxng")
                nc.vector.tensor_scalar_max(mxng[:, cs], mxn[:, cs], EPS)
                mrec = GA("mrec")
                nc.vector.reciprocal(mrec[:, cs], mxng[:, cs])
                cc = GA("cc")
                nc.vector.scalar_tensor_tensor(out=cc[:, cs],
                                               in0=mxn[:, cs], scalar=0.5,
                                               in1=rr2[:, cs],
                                               op0=OP.mult, op1=OP.mult)
                tch = GA("tch")
                nc.scalar.activation(tch[:, cs], cc[:, cs], AF.Tanh)
                tcg = GA("tcg")
                nc.vector.tensor_scalar_max(tcg[:, cs], tch[:, cs], EPS)
                tcrec = GA("tcrec")
                nc.vector.reciprocal(tcrec[:, cs], tcg[:, cs])
                psA = GA("psA")
                nc.vector.tensor_scalar(out=psA[:, cs], in0=tcrec[:, cs],
                                        scalar1=MN, scalar2=1.0,
                                        op0=OP.mult, op1=OP.min)
                sp0 = GA("sp0")
                nc.vector.tensor_tensor(out=sp0[:, cs], in0=tch[:, cs],
                                        in1=mrec[:, cs], op=OP.mult)
                spg = GA("spg")
                nc.vector.tensor_tensor(out=spg[:, cs], in0=sp0[:, cs],
                                        in1=psA[:, cs], op=OP.mult)
                tcm = GA("tcm")
                nc.vector.tensor_scalar_min(tcm[:, cs], tch[:, cs], MN)
                x2 = GA("x2")
                nc.vector.tensor_tensor(out=x2[:, cs], in0=tcm[:, cs],
                                        in1=tcm[:, cs], op=OP.mult)
                # pass 2: xy = sum((sp*mx) . y)
                xy = GA("xy")
                for t in trng:
                    jx = junkp.tile([P, D], f32, name="junk", tag="junk")
                    nc.vector.scalar_tensor_tensor(
                        out=jx[:], in0=agg_grid[:, tsl(t)],
                        scalar=spg[:, t:t + 1], in1=y_ap,
                        op0=OP.mult, op1=OP.mult,
                        accum_out=xy[:, t:t + 1])
                # stage 2
                t0 = GA("t0")
                nc.vector.tensor_scalar(out=t0[:, cs], in0=xy[:, cs],
                                        scalar1=2.0, scalar2=1.0,
                                        op0=OP.mult, op1=OP.add)
                ag = GA("ag")
                nc.vector.tensor_scalar_add(ag[:, cs], t0[:, cs],
                                            y2col[:, 0:1])
                d0 = GA("d0")
                nc.vector.tensor_scalar_mul(d0[:, cs], x2[:, cs],
                                            y2col[:, 0:1])
                d1 = GA("d1")
                nc.vector.tensor_tensor(out=d1[:, cs], in0=d0[:, cs],
                                        in1=t0[:, cs], op=OP.add)
                dg = GA("dg")
                nc.vector.tensor_scalar_max(dg[:, cs], d1[:, cs], EPS)
                dinv = GA("dinv")
                nc.vector.reciprocal(dinv[:, cs], dg[:, cs])
                alpha = GA("alpha")
                nc.vector.tensor_tensor(out=alpha[:, cs], in0=ag[:, cs],
                                        in1=dinv[:, cs], op=OP.mult)
                bsc = GA("bsc")
                nc.vector.tensor_scalar(out=bsc[:, cs], in0=x2[:, cs],
                                        scalar1=-1.0, scalar2=1.0,
                                        op0=OP.mult, op1=OP.add)
                beta = GA("beta")
                nc.vector.tensor_tensor(out=beta[:, cs], in0=bsc[:, cs],
                                        in1=dinv[:, cs], op=OP.mult)
                alphasp = GA("alphasp")
                nc.vector.tensor_tensor(out=alphasp[:, cs],
                                        in0=alpha[:, cs], in1=spg[:, cs],
                                        op=OP.mult)
                # pass 3: u = alphasp*mx + beta*y (into h_grid)
                un2 = GA("un2")
                for t in trng:
                    t1 = workp.tile([P, D], f32, name="t1", tag="t1")
                    nc.vector.tensor_scalar_mul(t1[:], y_ap,
                                                beta[:, t:t + 1])
                    us = h_grid[:, tsl(t)]
                    nc.vector.scalar_tensor_tensor(
                        out=us, in0=agg_grid[:, tsl(t)],
                        scalar=alphasp[:, t:t + 1], in1=t1[:],
                        op0=OP.mult, op1=OP.add)
                    ju = junkp.tile([P, D], f32, name="junk", tag="junk")
                    nc.scalar.activation(ju[:], us, AF.Square,
                                         accum_out=un2[:, t:t + 1])
                # stage 3: gamma
                un = GA("un")
                nc.scalar.activation(un[:, cs], un2[:, cs], AF.Sqrt)
                ung = GA("ung")
                nc.vector.tensor_scalar_max(ung[:, cs], un[:, cs], EPS)
                urec = GA("urec")
                nc.vector.reciprocal(urec[:, cs], ung[:, cs])
                h2n = GA("h2n")
                nc.vector.tensor_scalar_min(h2n[:, cs], un[:, cs], MN)
                at2u = artanh2h("s3", h2n, cs)
                h2ng = GA("h2ng")
                nc.vector.tensor_scalar_max(h2ng[:, cs], h2n[:, cs], EPS)
                hrec = GA("hrec")
                nc.vector.reciprocal(hrec[:, cs], h2ng[:, cs])
                lam2 = GA("lam2")
                nc.vector.tensor_tensor(out=lam2[:, cs], in0=at2u[:, cs],
                                        in1=hrec[:, cs], op=OP.mult)
                pst = GA("pst")
                nc.vector.tensor_scalar(out=pst[:, cs], in0=urec[:, cs],
                                        scalar1=MN, scalar2=1.0,
                                        op0=OP.mult, op1=OP.min)
                gm0 = GA("gm0")
                nc.vector.scalar_tensor_tensor(out=gm0[:, cs],
                                               in0=lam2[:, cs], scalar=0.5,
                                               in1=pst[:, cs],
                                               op0=OP.mult, op1=OP.mult)
                gam = GA("gam")
                nc.vector.tensor_tensor(out=gam[:, cs], in0=gm0[:, cs],
                                        in1=disg[:, cs], op=OP.mult)
                # ts tiles out
                for t in trng:
                    tst = workp.tile([P, D], EX, name="tst", tag="tst")
                    nc.vector.tensor_scalar_mul(tst[:],
                                                h_grid[:, tsl(t)],
                                                gam[:, t:t + 1])
                    nc.sync.dma_start(out=ts_loc[l][t * P:(t + 1) * P, :],
                                      in_=tst[:])

            def emit_AG(l):
                nc.gpsimd.collective_compute(
                    "AllGather", OP.bypass,
                    replica_groups=[list(range(NCORES))],
                    ins=[ts_loc[l].opt()], outs=[ts_full[l].opt()])

            bvt = {}

            def GB(tag):
                if tag not in bvt:
                    bvt[tag] = G(tag)
                return bvt[tag]

            def expmap_grid_cs(nm, n2, cs, with_dis=True):
                """sig2 columns cs of expmap0(dis*agg) incl. dst-side dis
                (or plain expmap0 scaling when with_dis=False);
                also writes hn2[:, cs]."""
                n = GB(nm + "_n")
                nc.scalar.activation(n[:, cs], n2[:, cs], AF.Sqrt)
                if with_dis:
                    npr = GB(nm + "_npr")
                    nc.vector.tensor_tensor(out=npr[:, cs], in0=n[:, cs],
                                            in1=disg[:, cs], op=OP.mult)
                else:
                    npr = n
                ng = GB(nm + "_ng")
                nc.vector.tensor_scalar_max(ng[:, cs], npr[:, cs], EPS)
                tn = GB(nm + "_tn")
                nc.scalar.activation(tn[:, cs], npr[:, cs], AF.Tanh)
                rec = GB(nm + "_rec")
                nc.vector.reciprocal(rec[:, cs], ng[:, cs])
                sc0 = GB(nm + "_sc0")
                nc.vector.tensor_tensor(out=sc0[:, cs], in0=tn[:, cs],
                                        in1=rec[:, cs], op=OP.mult)
                tng = GB(nm + "_tng")
                nc.vector.tensor_scalar_max(tng[:, cs], tn[:, cs], EPS)
                trec = GB(nm + "_trec")
                nc.vector.reciprocal(trec[:, cs], tng[:, cs])
                ps = GB(nm + "_ps")
                nc.vector.tensor_scalar(out=ps[:, cs], in0=trec[:, cs],
                                        scalar1=MN, scalar2=1.0,
                                        op0=OP.mult, op1=OP.min)
                sig = GB(nm + "_sig")
                nc.vector.tensor_tensor(out=sig[:, cs], in0=sc0[:, cs],
                                        in1=ps[:, cs], op=OP.mult)
                if with_dis:
                    sig2 = GB(nm + "_sig2")
                    nc.vector.tensor_tensor(out=sig2[:, cs], in0=sig[:, cs],
                                            in1=disg[:, cs], op=OP.mult)
                    sig = sig2
                tnm = GB(nm + "_tnm")
                nc.vector.tensor_scalar_min(tnm[:, cs], tn[:, cs], MN)
                nc.vector.tensor_tensor(out=hn2[:, cs], in0=tnm[:, cs],
                                        in1=tnm[:, cs], op=OP.mult)
                return sig

            GRP = 5  # blocks per epilogue group (activation-table batching)

            def emit_phaseB(l):
                an2 = G("an2")
                mtiles = {}
                for b in range(T):
                    S = sblkp.tile([P, C * P], EX, name="S", tag="S")
                    nc.vector.tensor_tensor(
                        out=S[:].rearrange("p (c j) -> p c j", c=C),
                        in0=edst_sb[:, b * C:(b + 1) * C].to_broadcast(
                            [P, C, P]),
                        in1=iota_sb[:].rearrange("p (o j) -> p o j", o=1)
                            .to_broadcast([P, C, P]),
                        op=OP.is_equal)
                    pa = psagp.tile([P, D], f32, name="pa", tag="pa")
                    for c in range(C):
                        j = b * C + c
                        g, s = divmod(j, CPG)
                        if g not in mtiles:
                            m = msgp.tile([P, CPG * D], EX, name="m", tag="m")
                            nc.gpsimd.dma_gather(
                                m[:].rearrange("p (c e) -> p c e", c=CPG),
                                ts_full[l],
                                gidx_sb[:, g * (GN // 16):(g + 1) * (GN // 16)],
                                GN, GN, D, queue_num=g % 4)
                            mtiles = {g: m}
                        m = mtiles[g]
                        nc.tensor.matmul(pa[:],
                                         lhsT=S[:, c * P:(c + 1) * P],
                                         rhs=m[:, s * D:(s + 1) * D],
                                         start=(c == 0), stop=(c == C - 1))
                    jj = junkp.tile([P, D], f32, name="junk", tag="junk")
                    nc.scalar.activation(jj[:], pa[:], AF.Square,
                                         accum_out=an2[:, b:b + 1])
                    # defer expmap scaling to the group epilogue
                    nc.scalar.copy(h_grid[:, tsl(b)], pa[:])
                    if (b + 1) % GRP == 0:
                        g0 = b + 1 - GRP
                        cs = slice(g0, b + 1)
                        sig = expmap_grid_cs("emB", an2, cs)
                        for t in range(g0, b + 1):
                            nc.vector.tensor_scalar_mul(
                                h_grid[:, tsl(t)], h_grid[:, tsl(t)],
                                sig[:, t:t + 1])
                        if l == 0:
                            for t in range(g0, b + 1):
                                emit_pass1(1, t)
                            if b == 14:
                                # embed layer-1 stage-A half 0 (tiles 0-9,
                                # all inputs ready since block 9) so only
                                # half 1 remains exposed after B0
                                emit_stageA(1, slice(0, T // 2),
                                            range(0, T // 2), first=True)
                        else:
                            for t in range(g0, b + 1):
                                nc.sync.dma_start(
                                    out=out_ext[t * P:(t + 1) * P, :],
                                    in_=h_grid[:, tsl(t)])

            # layer 0 phase A
            for t in range(T):
                emit_pass1(0, t)
            NH = 2 if T % 2 == 0 and T >= 2 else 1
            TH = T // NH
            for hh in range(NH):
                emit_stageA(0, slice(hh * TH, (hh + 1) * TH),
                            range(hh * TH, (hh + 1) * TH), first=(hh == 0))
            emit_AG(0)
            # layer 0 phase B (embeds layer-1 pass1 via group epilogues)
            emit_phaseB(0)
            # layer 1 phase A: half 0 was embedded in B0; finish half 1
            emit_stageA(1, slice(T // 2, T), range(T // 2, T), first=False)
            emit_AG(1)
            # layer 1 phase B
            emit_phaseB(1)

    nc.compile()
    return nc


def _get_program(T, C, NG, DC):
    key = (T, C, NG, DC)
    if key not in _prog_cache:
        _prog_cache[key] = _build_program(T, C, NG, DC)
    return _prog_cache[key]


# ----------------------------------------------------------------- entry

def run(inputs, trace=False, trace_kwargs=None):
    x = np.asarray(inputs["x"], np.float32)
    ei = np.asarray(inputs["edge_index"])
    W1 = np.asarray(inputs["W1"], np.float32)
    b1 = np.asarray(inputs["b1"], np.float32)
    W2 = np.asarray(inputs["W2"], np.float32)
    b2 = np.asarray(inputs["b2"], np.float32)
    N, D = x.shape
    assert D % P == 0
    meta, per_core = _host_prep(x, ei)
    T, C, NG, DC = meta["T"], meta["C"], meta["NG"], D // P
    n_loc, perm = meta["n_loc"], meta["perm"]

    wt = np.stack([np.ascontiguousarray(W1.T), np.ascontiguousarray(W2.T)])
    wt = wt.astype(ml_dtypes.bfloat16)
    y = np.stack([np.tile(_np_expmap0(b1)[None, :], (P, 1)),
                  np.tile(_np_expmap0(b2)[None, :], (P, 1))])

    nc = _get_program(T, C, NG, DC)
    in_maps = []
    for r in range(NCORES):
        m = dict(per_core[r])
        m["wt"] = wt
        m["y"] = y
        in_maps.append(m)

    kwargs = {}
    if trace:
        kwargs = dict(trace=True, trace_kwargs=trace_kwargs or {})
    res = run_bass_kernel_spmd(nc, in_maps, list(range(NCORES)), **kwargs)
    out = np.empty((N, D), np.float32)
    for r in range(NCORES):
        res_r = np.asarray(res.results[r]["out"])
        pr = perm[r]
        valid = pr >= 0
        out[r * n_loc + pr[valid]] = res_r[np.nonzero(valid)[0]]
    return out, res


def kernel(**inputs):
    out, _ = run(inputs)
    return out

